# revision 8
# baseline (speedup 1.0000x reference)
"""Trainium2 Bass kernel for nn_LinearLLM: out[b,t,v] = sum_{s>=t,w} x[b,s,w]*W[s,w,t,v] + bias.

Algebraic reduction: x[b,s,:] = embedding[src[b,s]] takes only V=6 values, so
the EMB=64 contraction is folded into the weight ON HOST:
    W2[(s,k),(t,v)] = sum_w emb[k,w] * weight[s,w,t,v] * mask(s>=t)
and the device computes a single one-hot matmul
    out[b,(t,v)] = sum_{(s,k)} onehot[b,(s,k)] * W2[(s,k),(t,v)]
with contraction K = L1*V = 3078 (25 chunks of 128) instead of L1*EMB = 32832.

Sharding: t-axis cyclic over 8 cores (core c owns t in {c, c+8, ...}) so the
causal prefix-width per K-chunk is uniform across cores -> one SPMD program.
Per-core DMA: ~0.7MB W2 slab + ~0.4MB one-hot + 0.1MB out (fp8) vs ~21MB for
the dense (s,w) formulation.

dtype: float8 e3m4 (4 mantissa bits), W2 pre-scaled by 64 so values land in
the normal range; one-hot 1.0 is exact in fp8. Measured end-to-end rel err
~1.4e-2 (vs 2e-2 tolerance). Set FP8=False for a bf16 fallback (~2e-3).

K-chunks are issued in DESCENDING order (24 first, full 390-col width with
start=True, so no zero-init matmul is needed).
"""
import numpy as np
import ml_dtypes

from concourse import bacc, tile
from concourse.bass_utils import run_bass_kernel_spmd
import concourse.mybir as mybir

B, L1, EMB, V, NCORES = 128, 513, 64, 6, 8
CNT = 65                       # padded t-count per core (core 0 has 65)
NCOLS = CNT * V                # 390 output columns per core
NROWS = L1 * V                 # 3078 contraction rows (s,k)
NCHUNK = 25                    # ceil(3078/128) K-chunks of 128
NROWS_PAD = NCHUNK * 128       # 3200

FP8 = True
if FP8:
    MM_DT = mybir.dt.float8e3
    NP_DT = ml_dtypes.float8_e3m4
    SCALE = 64.0
else:
    MM_DT = mybir.dt.bfloat16
    NP_DT = ml_dtypes.bfloat16
    SCALE = 1.0


def _width(j):
    """Masked column-prefix width for K-chunk j (core-0 worst case)."""
    s_max = min(L1 - 1, (128 * (j + 1) - 1) // V)
    return 6 * min(CNT, s_max // 8 + 1)


# DMA groups of K-chunks (descending chunk order within each group, so chunk
# 24 -- full width, start=True -- goes first). Each group is ONE merged DMA
# carrying [one-hot chunks | W2 slab chunks]; per-dma_start fixed cost ~2us
# dominates, so groups alternate between the two HWDGE rings (ACT=scalar,
# SP=sync) to overlap it.
GROUPS = [
    list(range(24, 22, -1)),   # chunks 24,23      -> scalar ring
    list(range(22, 16, -1)),   # 22..17            -> sync ring
    list(range(16, 8, -1)),    # 16..9             -> scalar ring
    list(range(8, -1, -1)),    # 8..0              -> sync ring
]
assert sorted(j for g in GROUPS for j in g) == list(range(NCHUNK))


def _group_width(chunks):
    return 128 * len(chunks) + sum(_width(j) for j in chunks)

_CACHE = {}


def _build():
    if "nc" in _CACHE:
        return _CACHE["nc"]
    nc = bacc.Bacc("TRN2", target_bir_lowering=False, debug=False,
                   num_devices=NCORES)
    g_dram = [nc.declare_dram_parameter(f"g{i}", [128, _group_width(g)],
                                        MM_DT, isOutput=False)
              for i, g in enumerate(GROUPS)]
    out_dram = nc.declare_dram_parameter("out", [128, NCOLS],
                                         mybir.dt.float16, isOutput=True)

    with tile.TileContext(nc) as tc:
        with (
            tc.tile_pool(name="op", bufs=1) as op,
            tc.tile_pool(name="psum", bufs=1, space="PSUM") as psp,
        ):
            ps = psp.tile([128, NCOLS], mybir.dt.float32)
            rings = [nc.scalar, nc.sync]
            tiles = []
            # issue every group DMA up front, alternating HWDGE rings
            for i, g in enumerate(GROUPS):
                t = op.tile([128, _group_width(g)], MM_DT)
                rings[i % 2].dma_start(t[:], g_dram[i][:])
                tiles.append(t)
            for i, g in enumerate(GROUPS):
                t = tiles[i]
                base = 128 * len(g)
                ok = 0
                for idx, j in enumerate(g):
                    wj = _width(j)
                    nc.tensor.matmul(ps[:, :wj],
                                     t[:, idx * 128:(idx + 1) * 128],
                                     t[:, base + ok:base + ok + wj],
                                     start=(j == NCHUNK - 1),
                                     stop=(j == 0))
                    ok += wj

            o = op.tile([128, NCOLS], mybir.dt.float16)
            half = 192
            nc.vector.tensor_copy(o[:, :half], ps[:, :half])
            nc.scalar.copy(o[:, half:], ps[:, half:])
            nc.gpsimd.dma_start(out_dram[:], o[:])

    nc.compile()
    _CACHE["nc"] = nc
    return nc


def _prep_inputs(src, embedding, weight):
    src = np.asarray(src)
    emb = np.asarray(embedding, dtype=np.float32)
    weight = np.asarray(weight, dtype=np.float32)

    # one-hot lhsT, layout oh[p, j*128 + b] = 1 iff src[b, r//6] == r%6
    # with r = 128j + p  (shared by all cores)
    oh = np.zeros((128, NROWS_PAD), np.float32)
    r = np.arange(L1)[None, :] * V + src            # (B, L1)
    p = r % 128
    cols = (r // 128) * 128 + np.arange(B)[:, None]
    oh[p.ravel(), cols.ravel()] = 1.0
    oh = oh.astype(NP_DT)

    # W2[(s,k), (t,v)] = sum_w emb[k,w] * weight[s,w,t,v]
    W2 = np.matmul(emb[None], weight.reshape(L1, EMB, L1 * V))  # (513, 6, 3078)
    W2 = W2.reshape(NROWS, L1 * V)
    svals = np.arange(NROWS) // V

    in_maps = []
    for c in range(NCORES):
        tvals = np.arange(c, L1, 8)
        cnt = len(tvals)
        cols_c = (tvals[:, None] * V + np.arange(V)[None, :]).ravel()
        Wc = W2[:, cols_c] * (svals[:, None] >= np.repeat(tvals, V)[None, :])
        Wp = np.zeros((NROWS_PAD, NCOLS), np.float32)
        Wp[:NROWS, :cnt * V] = Wc
        q = (Wp * SCALE).astype(NP_DT)
        in_map = {}
        for i, g in enumerate(GROUPS):
            blocks = [oh[:, 128 * j:128 * (j + 1)] for j in g]
            blocks += [q[128 * j:128 * (j + 1), :_width(j)] for j in g]
            in_map[f"g{i}"] = np.ascontiguousarray(
                np.concatenate(blocks, axis=1))
        in_maps.append(in_map)
    return in_maps


def _unshard(results, bias):
    full = np.zeros((B, L1, V), np.float32)
    for c in range(NCORES):
        cnt = len(range(c, L1, 8))
        oc = results[c]["out"].astype(np.float32).reshape(B, CNT, V)
        full[:, c::8, :] = oc[:, :cnt, :] / SCALE
    full += np.asarray(bias, dtype=np.float32)[None]
    return np.ascontiguousarray(full.transpose(0, 2, 1))


def kernel(src, embedding, weight, bias):
    nc = _build()
    in_maps = _prep_inputs(src, embedding, weight)
    res = run_bass_kernel_spmd(nc, in_maps, list(range(NCORES)))
    return _unshard(res.results, bias)


# revision 10
# speedup vs baseline: 2.9584x; 2.9584x over previous
"""Trainium2 Bass kernel for nn_LinearLLM: out[b,t,v] = sum_{s>=t,w} x[b,s,w]*W[s,w,t,v] + bias.

Algebraic reduction: x[b,s,:] = embedding[src[b,s]] takes only V=6 values, so
the EMB=64 contraction is folded into the weight ON HOST:
    W2[(s,k),(t,v)] = sum_w emb[k,w] * weight[s,w,t,v] * mask(s>=t)
and the device computes a single one-hot matmul
    out[b,(t,v)] = sum_{(s,k)} onehot[b,(s,k)] * W2[(s,k),(t,v)]
with contraction K = L1*V = 3078 (25 chunks of 128) instead of L1*EMB = 32832.

Sharding: t-axis cyclic over 8 cores (core c owns t in {c, c+8, ...}) so the
causal prefix-width per K-chunk is uniform across cores -> one SPMD program.
Per-core DMA: ~0.7MB W2 slab + ~0.4MB one-hot + 0.1MB out (fp8) vs ~21MB for
the dense (s,w) formulation.

dtype: float8 e3m4 (4 mantissa bits), W2 pre-scaled by 64 so values land in
the normal range; one-hot 1.0 is exact in fp8. Measured end-to-end rel err
~1.4e-2 (vs 2e-2 tolerance). Set FP8=False for a bf16 fallback (~2e-3).

K-chunks are issued in DESCENDING order (24 first, full 390-col width with
start=True, so no zero-init matmul is needed).
"""
import numpy as np
import ml_dtypes

from concourse import bacc, tile
from concourse.bass_utils import run_bass_kernel_spmd
import concourse.mybir as mybir

B, L1, EMB, V, NCORES = 128, 513, 64, 6, 8
CNT = 65                       # padded t-count per core (core 0 has 65)
NCOLS = CNT * V                # 390 output columns per core
NROWS = L1 * V                 # 3078 contraction rows (s,k)
NCHUNK = 25                    # ceil(3078/128) K-chunks of 128
NROWS_PAD = NCHUNK * 128       # 3200

FP8 = True
if FP8:
    MM_DT = mybir.dt.float8e3
    NP_DT = ml_dtypes.float8_e3m4
    SCALE = 64.0
else:
    MM_DT = mybir.dt.bfloat16
    NP_DT = ml_dtypes.bfloat16
    SCALE = 1.0


def _width(j):
    """Masked column-prefix width for K-chunk j (core-0 worst case)."""
    s_max = min(L1 - 1, (128 * (j + 1) - 1) // V)
    return 6 * min(CNT, s_max // 8 + 1)


# DMA groups of K-chunks (descending chunk order within each group, so chunk
# 24 -- full width, start=True -- goes first). Each group is ONE merged DMA
# carrying [one-hot chunks | W2 slab chunks]; per-dma_start fixed cost ~2us
# dominates, so groups alternate between the two HWDGE rings (ACT=scalar,
# SP=sync) to overlap it.
GROUPS = [
    list(range(24, 22, -1)),   # chunks 24,23      -> scalar ring
    list(range(22, 16, -1)),   # 22..17            -> sync ring
    list(range(16, 8, -1)),    # 16..9             -> scalar ring
    list(range(8, -1, -1)),    # 8..0              -> sync ring
]
assert sorted(j for g in GROUPS for j in g) == list(range(NCHUNK))


def _group_width(chunks):
    return 128 * len(chunks) + sum(_width(j) for j in chunks)

_CACHE = {}


def _build():
    if "nc" in _CACHE:
        return _CACHE["nc"]
    nc = bacc.Bacc("TRN2", target_bir_lowering=False, debug=False,
                   num_devices=NCORES)
    g_dram = [nc.declare_dram_parameter(f"g{i}", [128, _group_width(g)],
                                        MM_DT, isOutput=False)
              for i, g in enumerate(GROUPS)]
    out_dram = nc.declare_dram_parameter("out", [128, NCOLS],
                                         mybir.dt.float16, isOutput=True)

    with tile.TileContext(nc) as tc:
        with (
            tc.tile_pool(name="op", bufs=1) as op,
            tc.tile_pool(name="psum", bufs=1, space="PSUM") as psp,
        ):
            ps = psp.tile([128, NCOLS], mybir.dt.float32)
            tiles = []
            # issue every group DMA up front on the SWDGE (gpsimd) path:
            # its Q7 descriptor generation (~5ns/desc) feeds all 16 SDMA
            # engines, unlike HWDGE whose shared generator caps at ~65GB/s
            for i, g in enumerate(GROUPS):
                t = op.tile([128, _group_width(g)], MM_DT)
                nc.gpsimd.dma_start(t[:], g_dram[i][:])
                tiles.append(t)
            for i, g in enumerate(GROUPS):
                t = tiles[i]
                base = 128 * len(g)
                ok = 0
                for idx, j in enumerate(g):
                    wj = _width(j)
                    nc.tensor.matmul(ps[:, :wj],
                                     t[:, idx * 128:(idx + 1) * 128],
                                     t[:, base + ok:base + ok + wj],
                                     start=(j == NCHUNK - 1),
                                     stop=(j == 0))
                    ok += wj

            o = op.tile([128, NCOLS], mybir.dt.float16)
            nc.vector.tensor_copy(o[:], ps[:])
            nc.gpsimd.dma_start(out_dram[:], o[:])

    nc.compile()
    _CACHE["nc"] = nc
    return nc


def _prep_inputs(src, embedding, weight):
    src = np.asarray(src)
    emb = np.asarray(embedding, dtype=np.float32)
    weight = np.asarray(weight, dtype=np.float32)

    # one-hot lhsT, layout oh[p, j*128 + b] = 1 iff src[b, r//6] == r%6
    # with r = 128j + p  (shared by all cores)
    oh = np.zeros((128, NROWS_PAD), np.float32)
    r = np.arange(L1)[None, :] * V + src            # (B, L1)
    p = r % 128
    cols = (r // 128) * 128 + np.arange(B)[:, None]
    oh[p.ravel(), cols.ravel()] = 1.0
    oh = oh.astype(NP_DT)

    # W2[(s,k), (t,v)] = sum_w emb[k,w] * weight[s,w,t,v]
    W2 = np.matmul(emb[None], weight.reshape(L1, EMB, L1 * V))  # (513, 6, 3078)
    W2 = W2.reshape(NROWS, L1 * V)
    svals = np.arange(NROWS) // V

    in_maps = []
    for c in range(NCORES):
        tvals = np.arange(c, L1, 8)
        cnt = len(tvals)
        cols_c = (tvals[:, None] * V + np.arange(V)[None, :]).ravel()
        Wc = W2[:, cols_c] * (svals[:, None] >= np.repeat(tvals, V)[None, :])
        Wp = np.zeros((NROWS_PAD, NCOLS), np.float32)
        Wp[:NROWS, :cnt * V] = Wc
        q = (Wp * SCALE).astype(NP_DT)
        in_map = {}
        for i, g in enumerate(GROUPS):
            blocks = [oh[:, 128 * j:128 * (j + 1)] for j in g]
            blocks += [q[128 * j:128 * (j + 1), :_width(j)] for j in g]
            in_map[f"g{i}"] = np.ascontiguousarray(
                np.concatenate(blocks, axis=1))
        in_maps.append(in_map)
    return in_maps


def _unshard(results, bias):
    full = np.zeros((B, L1, V), np.float32)
    for c in range(NCORES):
        cnt = len(range(c, L1, 8))
        oc = results[c]["out"].astype(np.float32).reshape(B, CNT, V)
        full[:, c::8, :] = oc[:, :cnt, :] / SCALE
    full += np.asarray(bias, dtype=np.float32)[None]
    return np.ascontiguousarray(full.transpose(0, 2, 1))


def kernel(src, embedding, weight, bias):
    nc = _build()
    in_maps = _prep_inputs(src, embedding, weight)
    res = run_bass_kernel_spmd(nc, in_maps, list(range(NCORES)))
    return _unshard(res.results, bias)
